# revision 9
# baseline (speedup 1.0000x reference)
"""Sparse (masked) multi-head attention on 8 Trainium2 NeuronCores.

Problem: nodes [2,2048,512], edge_mask [2,2048,2048] (bool),
q/kv/o linear layers with H=8 heads of DH=64.

Sharding: batch x head-group.  Core c handles batch b = c//4 and head group
g = c%4 (heads 2g, 2g+1 = inner columns g*128:(g+1)*128).  Each core
computes its two heads' attention over the full sequence plus its partial
contribution to the output projection; the host sums the 4 partials per
batch and adds bo.

Per-core dataflow (all matmuls bf16 inputs, fp32 PSUM accumulation):
  qT/kT [dh=128, N]  = wq_sliceT @ nodesT (+bias)        (dh on partitions)
  v     [N, dh=128]  = nodesT.T @ wv_slice
  per (io, h, jb): simT[j,i] = kT_h.T @ qT_h             (j on partitions)
              PT = exp(simT * DH**-0.5)   (ScalarE, free scale, bf16 out)
              PT *= maskT                  (DVE bf16 2x)
              numT[0:64,i] / den[64,i] = [v_h | 1].T @ PT  (ones col -> den)
  per (io, h) phase end: attnT_h = numT * recip(den)
  out[i,:] += attnT.T @ wo_slice           (contraction over both heads)

ScalarE is the critical engine (64 exps of [128,1024] from PSUM; measured
steady-state period ~1.2us => ~77us stream floor; bf16 PSUM matmul output
that would allow merged exps is TRN3-only).  v2 schedules around that
floor:
  - stream order is h-major (io, h, jb): only ONE num PSUM tag is filling
    at a time and each phase's softmax chain (recip/broadcast/normalize)
    drains while the NEXT phase streams, so no i-half boundary bubble.
  - loads are critical-path-first over FOUR rings (scalar/sync HWDGE for
    weights+nodesT, gpsimd SWDGE + vector HWDGE for the i-half0 mask,
    sync again for the i-half1 mask) -- the first exp fires at ~14us
    instead of ~29us, and the mask rings stay ~2 chunks ahead of the
    2 x 0.25MB/1.2us consumption rate.
  - AV emission is interleaved 1:1 with sim/exp steps at a fixed depth
    (PIPE=10) so the in-order PE queue never bursts AVs and never parks
    a blocked AV in front of pending sims.
  - v/k-h1/q-h1 projections are woven into the early-stream PE slack
    (steps with no AVs yet), borrowing the num PSUM tags strictly before
    each tag's phase allocation (no cyclic slot dependencies).
  - the whole output projection runs in the tail: chains split in half-i
    pieces for latency, o-proj matmuls pipeline through the two freed num
    tags (2 sub-slots each) with copies alternating Scalar/Vector (both
    idle post-stream), output DMA spread over sync/scalar/gpsimd rings.
"""
import numpy as np
import ml_dtypes

import concourse.bass as bass
import concourse.bacc as bacc
import concourse.tile as tile
from concourse import mybir
from concourse.bass_utils import run_bass_kernel_spmd
from bass_rust import add_dep_helper

B, N, DIM = 2, 2048, 512
H, DH = 8, 64
INNER = H * DH
SCALE = DH ** -0.5
NCORES = 8
HEADS_PER_CORE = 2
HG = 128            # inner columns per core (2 heads x 64)
NJB = N // 128      # 16 j-blocks
NC_DIM = DIM // 128  # 4 contraction chunks over DIM
NH = N // 2          # 1024-column i-half

BF16 = mybir.dt.bfloat16
F32 = mybir.dt.float32
ts = bass.ts
ds = bass.ds

WARMUP_MM = 20       # dummy matmuls to keep PE HAM-warm through the loads
PIPE = 10            # AV steps trailing the sim/exp stream


def _build():
    nc = bacc.Bacc(monotonic_sem_count=0)
    nT_d = nc.declare_dram_parameter("nodesT", [DIM, N], BF16, isOutput=False)
    maskT_d = nc.declare_dram_parameter("maskT", [N, N], BF16, isOutput=False)
    wq_d = nc.declare_dram_parameter("wq_s", [DIM, HG], BF16, isOutput=False)
    wk_d = nc.declare_dram_parameter("wk_s", [DIM, HG], BF16, isOutput=False)
    wv_d = nc.declare_dram_parameter("wv_s", [DIM, HG], BF16, isOutput=False)
    wo_d = nc.declare_dram_parameter("wo_s", [HG, DIM], BF16, isOutput=False)
    bq_d = nc.declare_dram_parameter("bq_s", [HG, 1], F32, isOutput=False)
    bk_d = nc.declare_dram_parameter("bk_s", [HG, 1], F32, isOutput=False)
    out_d = nc.declare_dram_parameter("out", [N, DIM], BF16, isOutput=True)

    with tile.TileContext(nc) as tc:
        with (
            tc.tile_pool(name="persist", bufs=1) as persist,
            tc.tile_pool(name="ptp", bufs=20) as ptp,
            tc.tile_pool(name="denp", bufs=1) as denp,
            tc.tile_pool(name="outp", bufs=4) as outp,
            # PSUM (8 banks): psA = 2 rotating sim slots [128,1024]f32
            # (2 banks each); psB = num0/num1 tags [65,1024]f32 (2 banks
            # each), alternating per (io,h) phase.  Warmup, the v/q1/k1
            # projections and the tail o-proj borrow the num tags, always
            # strictly before (or after) the tag's phase allocation.
            tc.tile_pool(name="psA", bufs=2, space="PSUM") as psA,
            tc.tile_pool(name="psB", bufs=1, space="PSUM") as psB,
        ):
            # ---- loads, critical-path-first ----
            # scalar ring: wq, bq, nT c0/c1 (half0 first) -- all issued on
            # ScalarE before the exp stream exists.
            wq = persist.tile([128, NC_DIM, HG], BF16)
            nc.scalar.dma_start(
                out=wq[:], in_=wq_d.rearrange("(c p) m -> p c m", p=128)
            )
            bq = persist.tile([HG, 1], F32)
            nc.scalar.dma_start(out=bq[:], in_=bq_d[:])
            wk = persist.tile([128, NC_DIM, HG], BF16)
            nc.sync.dma_start(
                out=wk[:], in_=wk_d.rearrange("(c p) m -> p c m", p=128)
            )
            bk = persist.tile([HG, 1], F32)
            nc.sync.dma_start(out=bk[:], in_=bk_d[:])

            nT = persist.tile([128, NC_DIM, N], BF16)
            nT_r = nT_d.rearrange("(c p) n -> p c n", p=128)
            wv = persist.tile([128, NC_DIM, HG], BF16)
            wv_r = wv_d.rearrange("(c p) m -> p c m", p=128)
            # scalar ring: nT c0/c1, half0 then half1
            for nh in range(2):
                for c in (0, 1):
                    nc.scalar.dma_start(
                        out=nT[:, c, ts(nh, NH)], in_=nT_r[:, c, ts(nh, NH)]
                    )
            # sync ring: nT c2/c3 half0 interleaved with wv (v-proj c-chunks
            # become ready as their nT chunks land), then nT c2/c3 half1, wo.
            nc.sync.dma_start(out=nT[:, 2, ts(0, NH)], in_=nT_r[:, 2, ts(0, NH)])
            nc.sync.dma_start(out=wv[:, 0:2, :], in_=wv_r[:, 0:2, :])
            nc.sync.dma_start(out=nT[:, 3, ts(0, NH)], in_=nT_r[:, 3, ts(0, NH)])
            nc.sync.dma_start(out=wv[:, 2:4, :], in_=wv_r[:, 2:4, :])
            nc.sync.dma_start(out=nT[:, 2, ts(1, NH)], in_=nT_r[:, 2, ts(1, NH)])
            nc.sync.dma_start(out=nT[:, 3, ts(1, NH)], in_=nT_r[:, 3, ts(1, NH)])
            wo = persist.tile([HG, DIM], BF16)
            nc.sync.dma_start(out=wo[:], in_=wo_d[:])

            # mask, split by first-consumption time across the three rings
            # (only gpsimd/SP/Activation can initiate DMAs).  i-half0 is
            # consumed from the stream start at ~0.25MB/1.2us: jb0-7 on the
            # gpsimd SWDGE ring (starts immediately), jb8-11 on the scalar
            # ring and jb12-15 on the sync ring (both queue behind the nT
            # pieces and land in the low-20s us).  i-half1 (consumed from
            # ~t+40us) splits jb0-7 on sync (behind wo) / jb8-15 on scalar.
            maskT = persist.tile([128, NJB, N], BF16)
            maskT_r = maskT_d.rearrange("(g p) i -> p g i", p=128)
            # PE warm-up seed is DVE's first instruction so the dummy-matmul
            # chain (emitted below) starts as early as possible.
            wrm_src = persist.tile([128, 512], BF16)
            nc.vector.memset(wrm_src[:], 0.0)
            for lo, sz in ((0, 2), (2, 2), (4, 2), (6, 2)):
                nc.gpsimd.dma_start(
                    out=maskT[:, ds(lo, sz), ds(0, NH)],
                    in_=maskT_r[:, ds(lo, sz), ds(0, NH)],
                )
            nc.scalar.dma_start(
                out=maskT[:, ds(8, 4), ds(0, NH)],
                in_=maskT_r[:, ds(8, 4), ds(0, NH)],
            )
            nc.scalar.dma_start(
                out=maskT[:, ds(8, 8), ds(NH, NH)],
                in_=maskT_r[:, ds(8, 8), ds(NH, NH)],
            )
            nc.sync.dma_start(
                out=maskT[:, ds(12, 4), ds(0, NH)],
                in_=maskT_r[:, ds(12, 4), ds(0, NH)],
            )
            for lo in (0, 4):
                nc.sync.dma_start(
                    out=maskT[:, ds(lo, 4), ds(NH, NH)],
                    in_=maskT_r[:, ds(lo, 4), ds(NH, NH)],
                )

            # v-row ones columns (den accumulators); GpSimd after its DMA
            # issues, long before the first AV.
            v_sb = persist.tile([128, NJB, 130], BF16)
            nc.gpsimd.memset(v_sb[:, :, 64:65], 1.0)
            nc.gpsimd.memset(v_sb[:, :, 129:130], 1.0)

            # ---- PE warm-up: dummy matmul chain spanning the load phase so
            # the first projections run at the warm 2.4 GHz clock.  Nothing
            # reads the PSUM result (next tag user overwrites, start=True).
            wrm_ps = psB.tile([128, NH], F32, tag="num0")
            for i in range(WARMUP_MM):
                nc.tensor.matmul(
                    wrm_ps[:, 0:512], lhsT=wrm_src[:, 0:128], rhs=wrm_src[:],
                    start=(i == 0), stop=(i == WARMUP_MM - 1),
                )

            # ---- q/k projections ----
            qT = persist.tile([128, N], BF16)
            # kTz[:, h, :]: head h's dh rows at their original partitions,
            # the other head's rows zero -- sim matmuls contract over all
            # 128 partitions.  (columns of qT are i; columns of kTz are j)
            kTz = persist.tile([128, 2, N], BF16)
            nc.vector.memset(kTz[:], 0.0)

            def proj(which, half, pps):
                w_t = wq if which == "q" else wk
                for isl in range(2):
                    for c in range(NC_DIM):
                        nc.tensor.matmul(
                            pps[:, ts(isl, 512)],
                            lhsT=w_t[:, c, :],
                            rhs=nT[:, c, ts(half * 2 + isl, 512)],
                            start=(c == 0),
                            stop=(c == NC_DIM - 1),
                        )
                if which == "q" and half == 0:
                    # ScalarE is free before the exp stream starts
                    nc.scalar.activation(
                        out=qT[:, ts(half, NH)], in_=pps[:],
                        func=mybir.ActivationFunctionType.Identity, bias=bq[:],
                    )
                elif which == "q":
                    # mid-stream: DVE, so the exp queue is never interrupted
                    nc.vector.tensor_scalar(
                        out=qT[:, ts(half, NH)], in0=pps[:],
                        scalar1=bq[:], scalar2=None,
                        op0=mybir.AluOpType.add,
                    )
                else:
                    nc.vector.tensor_scalar(
                        out=kTz[0:64, 0, ts(half, NH)], in0=pps[0:64, :],
                        scalar1=bk[0:64, :], scalar2=None,
                        op0=mybir.AluOpType.add,
                    )
                    nc.vector.tensor_scalar(
                        out=kTz[64:128, 1, ts(half, NH)], in0=pps[64:128, :],
                        scalar1=bk[64:128, :], scalar2=None,
                        op0=mybir.AluOpType.add,
                    )

            # q and k half-0 in the two rotating psA slots so their
            # PSUM->SBUF copies (ScalarE / DVE) run concurrently.
            qpps = psA.tile([128, NH], F32, tag="sim")
            proj("q", 0, qpps)
            kpps = psA.tile([128, NH], F32, tag="sim")
            proj("k", 0, kpps)

            def emit_v_proj(jb_lo, jb_hi):
                for jb in range(jb_lo, jb_hi):
                    vps = psB.tile([128, HG], F32, tag=f"num{jb % 2}")
                    for c in range(NC_DIM):
                        nc.tensor.matmul(
                            vps[:],
                            lhsT=nT[:, c, ts(jb, 128)],
                            rhs=wv[:, c, :],
                            start=(c == 0),
                            stop=(c == NC_DIM - 1),
                        )
                    nc.vector.tensor_copy(
                        v_sb[:, jb, 0:130].rearrange(
                            "p (h c) -> p h c", h=2
                        )[:, :, 0:64],
                        vps[:].rearrange("p (h c) -> p h c", h=2),
                    )

            attnT = persist.tile([128, N], BF16)

            def sim_exp_mask(io, h, jb):
                sps = psA.tile([128, NH], F32, tag="sim")
                for isl in range(2):
                    nc.tensor.matmul(
                        sps[:, ts(isl, 512)],
                        lhsT=kTz[:, h, ts(jb, 128)],
                        rhs=qT[:, ds(io * NH + isl * 512, 512)],
                        start=True,
                        stop=True,
                    )
                pt = ptp.tile([128, NH], BF16, tag="pt")
                nc.scalar.activation(
                    out=pt[:],
                    in_=sps[:],
                    func=mybir.ActivationFunctionType.Exp,
                    scale=SCALE,
                )
                nc.vector.tensor_mul(pt[:], pt[:], maskT[:, jb, ds(io * NH, NH)])
                return pt

            # softmax epilogue per (io, h) phase, split in half-i pieces to
            # cut serial latency: recip of the den row straight from PSUM
            # (DVE), one partition-broadcast per piece (GpSimd), normalize
            # on DVE.  Single rec tiles per chain; disjoint piece slices
            # overlap via subtile deps.
            def chain(io, h, nps):
                # den must be staged in SBUF: reciprocal_approx_fast's
                # BITWISE_NOT exponent-flip seed misbehaves on a PSUM read
                # on HW (CoreSim models it fine; plain DVE ops reading PSUM
                # are fine -- the normalize below reads nps directly).
                den1 = denp.tile([1, 2, 512], F32, tag="den1")
                rec1 = denp.tile([1, 2, 512], F32, tag="rec1")
                recb = denp.tile([64, 2, 512], F32, tag="recb")
                for piece in range(2):
                    sl = ds(piece * 512, 512)
                    nc.vector.tensor_copy(den1[:, piece, :], nps[64:65, sl])
                    nc.vector.reciprocal_approx_fast(
                        out=rec1[:, piece, :], in_=den1[:, piece, :]
                    )
                    nc.gpsimd.partition_broadcast(
                        recb[:, piece, :], rec1[:, piece, :]
                    )
                    nc.vector.tensor_mul(
                        attnT[ts(h, 64), ds(io * NH + piece * 512, 512)],
                        nps[0:64, sl],
                        recb[:, piece, :],
                    )

            # ---- attention stream: h-major software pipeline ----
            from collections import deque

            steps = [
                (io, h, jb)
                for io in range(2)
                for h in range(HEADS_PER_CORE)
                for jb in range(NJB)
            ]
            pending = deque()
            num_of = {}

            def av(io, h, jb, pt):
                for isl in range(2):
                    nc.tensor.matmul(
                        num_of[(io, h)][:, ts(isl, 512)],
                        lhsT=v_sb[:, jb, ts(h, 65)],
                        rhs=pt[:, ts(isl, 512)],
                        start=(jb == 0),
                        stop=(jb == NJB - 1),
                    )

            def pop_one():
                pio, ph, pjb, ppt = pending.popleft()
                if pjb == 0:
                    phase = 2 * pio + ph
                    num_of[(pio, ph)] = psB.tile(
                        [65, NH], F32, tag=f"num{phase % 2}",
                        name=f"num_{pio}_{ph}",
                    )
                av(pio, ph, pjb, ppt)
                if pjb == NJB - 1:
                    chain(pio, ph, num_of[(pio, ph)])

            for io, h, jb in steps:
                pt = sim_exp_mask(io, h, jb)
                pending.append((io, h, jb, pt))
                if (io, h, jb) == (0, 0, 1):
                    # early-stream PE slack (no AVs yet): j-half0 of the
                    # v-projection (needs wv c-chunks + nT half0 only)
                    emit_v_proj(0, 8)
                if (io, h, jb) == (0, 0, 5):
                    # k j-half1 (needed by sim jb8) once nT half1 landed
                    kpps1 = psB.tile([128, NH], F32, tag="num1")
                    proj("k", 1, kpps1)
                if (io, h, jb) == (0, 0, 6):
                    emit_v_proj(8, NJB)
                if (io, h, jb) == (0, 0, 15):
                    # q i-half1 (needed by the io=1 sims at ~mid-stream)
                    qpps1 = psB.tile([128, NH], F32, tag="num1")
                    proj("q", 1, qpps1)
                while len(pending) > PIPE:
                    pop_one()
            while pending:
                pop_one()

            # ---- output projection, all in the tail ----
            # ScalarE and DVE are free after the last exp/chain; copies
            # alternate between them.  MMs pipeline through the freed num
            # tags: blocks 0-7 (i-half0, attnT ready since chain(0,*))
            # through num0's two sub-slots while chain(1,1) drains, blocks
            # 8-15 across both tags (4 sub-slots).  Output DMA per 2-block
            # chunk over sync/scalar/gpsimd rings.
            out_r = out_d.rearrange("(g p) m -> p g m", p=128)
            ops0 = psB.tile([128, 2, 512], F32, tag="num0")
            ops1 = None
            dma_engs = [nc.sync, nc.scalar, nc.gpsimd, nc.sync]
            for pair in range(8):
                osb = outp.tile([128, 2, DIM], BF16, tag="osb")
                for k in range(2):
                    ib = pair * 2 + k
                    if ib == 8:
                        ops1 = psB.tile([128, 2, 512], F32, tag="num1")
                    if ib < 8:
                        ops = ops0[:, ib % 2, :]
                    else:
                        ops = (ops0 if (ib // 2) % 2 == 0 else ops1)[:, ib % 2, :]
                    nc.tensor.matmul(
                        ops, lhsT=attnT[:, ts(ib, 128)], rhs=wo[:],
                        start=True, stop=True,
                    )
                    if ib % 2 == 0:
                        nc.scalar.copy(out=osb[:, k, :], in_=ops)
                    else:
                        nc.vector.tensor_copy(osb[:, k, :], ops)
                dma_engs[pair % 4].dma_start(
                    out=out_r[:, ds(pair * 2, 2), :], in_=osb[:]
                )

    # Bacc.compile runs generate_event_semaphores, which splits multi-sem
    # waits down to the 1-wait-per-instruction limit walrus enforces.
    nc.compile()

    # Bacc's dce_regs leaves the (unread) engine-preamble register writes
    # behind at this kernel size, with deferred reg_id=-1 -- walrus then
    # fails "Reg has not been allocated yet".  Nothing reads them, so any
    # valid unique per-engine id works.
    from collections import defaultdict

    next_id = defaultdict(lambda: 8)
    for a in nc.m.functions[0].allocations:
        if type(a).__name__ == "Register" and a.reg_id == -1:
            a.reg_id = next_id[str(a.engine)]
            next_id[str(a.engine)] += 1
    return nc


_NC_CACHE = None


def _get_nc():
    global _NC_CACHE
    if _NC_CACHE is None:
        _NC_CACHE = _build()
    return _NC_CACHE


def _prep_in_maps(nodes, edge_mask, wq, bq, wkv, bkv, wo, bo):
    bf16 = ml_dtypes.bfloat16
    wk_full, wv_full = wkv[:, :INNER], wkv[:, INNER:]
    bk_full = bkv[:INNER]
    per_batch = []
    for b in range(B):
        per_batch.append(
            (
                np.ascontiguousarray(nodes[b].T).astype(bf16),
                np.ascontiguousarray(edge_mask[b].T).astype(bf16),
            )
        )
    in_maps = []
    for core in range(NCORES):
        b, g = core // 4, core % 4
        cs = slice(g * HG, (g + 1) * HG)
        nT_b, maskT_b = per_batch[b]
        in_maps.append(
            {
                "nodesT": nT_b,
                "maskT": maskT_b,
                "wq_s": np.ascontiguousarray(wq[:, cs]).astype(bf16),
                "wk_s": np.ascontiguousarray(wk_full[:, cs]).astype(bf16),
                "wv_s": np.ascontiguousarray(wv_full[:, cs]).astype(bf16),
                "wo_s": np.ascontiguousarray(wo[cs, :]).astype(bf16),
                "bq_s": np.ascontiguousarray(bq[cs]).reshape(HG, 1).astype(np.float32),
                "bk_s": np.ascontiguousarray(bk_full[cs]).reshape(HG, 1).astype(np.float32),
            }
        )
    return in_maps


def kernel(nodes, edge_mask, wq, bq, wkv, bkv, wo, bo, _trace=False, _trace_kwargs=None):
    nodes = np.asarray(nodes, dtype=np.float32)
    edge_mask = np.asarray(edge_mask)
    wq = np.asarray(wq, dtype=np.float32)
    bq = np.asarray(bq, dtype=np.float32)
    wkv = np.asarray(wkv, dtype=np.float32)
    bkv = np.asarray(bkv, dtype=np.float32)
    wo = np.asarray(wo, dtype=np.float32)
    bo = np.asarray(bo, dtype=np.float32)

    nc = _get_nc()
    in_maps = _prep_in_maps(nodes, edge_mask, wq, bq, wkv, bkv, wo, bo)
    kw = {}
    if _trace:
        kw = dict(trace=True, **(_trace_kwargs or {}))
    res = run_bass_kernel_spmd(nc, in_maps, list(range(NCORES)), **kw)
    out = np.zeros((B, N, DIM), np.float32)
    for core in range(NCORES):
        out[core // 4] += res.results[core]["out"].astype(np.float32)
    # v-bias shifts each head's attention output by exactly bv (softmax
    # weights sum to 1), so its output contribution is the constant bv @ wo.
    bv_full = bkv[INNER:]
    out += (bv_full @ wo + bo)[None, None, :]
    if _trace:
        return out, res
    return out
